# revision 30
# baseline (speedup 1.0000x reference)
"""Trainium2 Bass kernel for nn_ConvTran (conv stem + eRPE transformer + GAP).

Sharding: pure data parallel. B=16 split as 2 batches per core across 8 cores.
All parameters replicated; per-core outputs concatenated on host.

Design notes:
- All matmuls in bf16 (PSUM accumulates fp32).
- conv1 row-tiled 4x (tile_position), BN bias folded via ones-row (K=9);
  conv2 accumulation interleaved per tap-slot so it hides under the
  conv1 gelu stream.
- Attention: scores row-tiled 4x, pv/bv col-tiled 4x; exp over [128,1024]
  double-buffered PSUM; the attention PE stream is pure matmuls (softmax
  normalize/bias-add and output transposes deferred to phase L).
- 3 act-table loads total (Gelu -> Exp -> Sqrt); copies/relu/bias-adds
  placed on the Act engine only in phases where it would idle.
- Phase L is stage-major across both batches (one Act round trip per LN
  stage for all 16 chunks).
"""

import os
import numpy as np
import ml_dtypes

KDBG = bool(os.environ.get("KDBG"))
KDBG_G = int(os.environ.get("KDBG_G", "0"))

# ---- problem constants (hardcoded; kernel.py must be self-contained) ----
B, S, C_IN, E, H, DFF = 16, 1024, 4, 128, 8, 512
C1 = E * 4          # 512
DH = E // H         # 16
EPS = 1e-5
SCALE = float(E) ** -0.5
N_CORES = 8
NB = B // N_CORES   # batches per core = 2
NG = 2              # head groups of 4
SC = S // 128       # 8 s-chunks
JC = S // 128       # 8 j-chunks
F32 = np.float32
BF16 = ml_dtypes.bfloat16


class _Pack:
    """Column-packed [128, N] constant store."""

    def __init__(self, dtype):
        self.cols = []
        self.index = {}
        self.n = 0
        self.dtype = dtype

    def add(self, name, arr2d):
        a = np.zeros((128, arr2d.shape[1]), self.dtype)
        a[:arr2d.shape[0]] = arr2d
        self.index[name] = (self.n, arr2d.shape[1])
        self.cols.append(a)
        self.n += arr2d.shape[1]

    def finalize(self):
        return np.ascontiguousarray(np.concatenate(self.cols, axis=1))


def _host_prep(inp):
    wp = _Pack(BF16)   # bf16 matmul weights
    fp = _Pack(F32)    # fp32 scalars / bias vectors
    f = lambda a: np.asarray(a, dtype=F32)

    # conv1: folded BN scale into weights; bias as 9th contraction row.
    w1 = f(inp["conv1_w"][:, 0, 0, :])                 # [C1, 8]
    sA = f(inp["bn1_g"]) / np.sqrt(f(inp["bn1_v"]) + EPS)
    bA = (f(inp["conv1_b"]) - f(inp["bn1_m"])) * sA + f(inp["bn1_b"])
    w1s = w1 * sA[:, None]
    w1c4 = np.zeros((128, 128), F32)
    for cc in range(4):
        w1c4[32 * cc:32 * cc + 8, :] = w1s[cc * 128:(cc + 1) * 128, :].T
        w1c4[32 * cc + 8, :] = bA[cc * 128:(cc + 1) * 128]
    wp.add("w1c4", w1c4)

    # conv2: [128(c1), 16(k=r*4+cc), 128(e)]
    w2 = f(inp["conv2_w"])[:, :, :, 0]                 # [E, C1, 4]
    w2cT = np.zeros((128, 16, 128), F32)
    for r in range(4):
        for cc in range(4):
            w2cT[:, r * 4 + cc, :] = w2[:, cc * 128:(cc + 1) * 128, r].T
    wp.add("w2cT", w2cT.reshape(128, 16 * 128))
    sB = f(inp["bn2_g"]) / np.sqrt(f(inp["bn2_v"]) + EPS)
    fp.add("scaleB", sB[:, None])
    fp.add("biasB", ((f(inp["conv2_b"]) - f(inp["bn2_m"])) * sB
                     + f(inp["bn2_b"]))[:, None])

    # tAPE positional encoding, transposed [E, S]
    pos = np.arange(S, dtype=np.float64)[:, None]
    div = np.exp(np.arange(0, E, 2, dtype=np.float64) * (-np.log(10000.0) / E))
    ang = pos * div * (E / S)
    pe = np.zeros((S, E), np.float64)
    pe[:, 0::2] = np.sin(ang)
    pe[:, 1::2] = np.cos(ang)
    wp.add("peT", pe.astype(F32).T)

    # q/k weights, padded head layout [128, g*128 + 32c + dh]
    def pad_qk(w):
        w = f(w)
        wt = np.zeros((128, NG * 128), F32)
        for g in range(NG):
            for c in range(4):
                h = 4 * g + c
                wt[:, g * 128 + 32 * c:g * 128 + 32 * c + DH] = \
                    w[h * DH:(h + 1) * DH, :].T
        return wt
    wp.add("wqT", pad_qk(inp["wq"]))
    wp.add("wkT", pad_qk(inp["wk"]))
    wp.add("wvT", f(inp["wv"]).T)

    wp.add("ffw1T", f(inp["ff_w1"]).T)
    fp.add("ffb1", f(inp["ff_b1"]).reshape(4, 128).T)
    wp.add("ffw2T", f(inp["ff_w2"]).T.reshape(4, 128, 128)
           .transpose(1, 0, 2).reshape(128, 512))
    fp.add("ffb2", f(inp["ff_b2"])[:, None])

    m = np.arange(128)
    wp.add("bcast4", (m[None, :] // 32 == np.arange(4)[:, None]).astype(F32))
    wp.add("ident", np.eye(128, dtype=F32))
    wp.add("ones", np.ones((128, 1), F32))
    fp.add("eps", np.full((128, 1), EPS, F32))

    lnG = np.stack([f(inp["ln_attn_g"]), f(inp["ln1_g"]), f(inp["ln2_g"])])
    lnB = np.stack([f(inp["ln_attn_b"]), f(inp["ln1_b"]), f(inp["ln2_b"])])
    ln_identity = bool(np.allclose(lnG, 1.0) and np.allclose(lnB, 0.0))
    fp.add("lnG", np.broadcast_to(lnG.reshape(1, 3 * 128), (128, 384)).copy())
    fp.add("lnB", np.broadcast_to(lnB.reshape(1, 3 * 128), (128, 384)).copy())

    d = {"wpack": wp.finalize(), "fpack": fp.finalize()}

    # rel_bias diagonal store (bf16): T[jj, h, c] = rel[127 + c - jj, h]
    rel = f(inp["rel_bias"])                          # [2047, 8]
    jj = np.arange(128)[:, None]
    cidx = np.arange(1920)[None, :]
    ts = rel[127 + cidx - jj, :]                      # [128, 1920, 8]
    d["tstore"] = np.ascontiguousarray(
        ts.transpose(0, 2, 1).astype(BF16))           # [128, 8, 1920]
    return d, wp.index, fp.index, ln_identity


def _build_bass(widx, fidx, nw, nf, ln_identity):
    import concourse.bass as bass
    import concourse.bacc as bacc
    import concourse.tile as tile
    import concourse.mybir as mybir

    dt = mybir.dt
    AF = mybir.ActivationFunctionType
    ALU = mybir.AluOpType

    nc = bacc.Bacc("TRN2")

    xin = nc.dram_tensor("rhs8", [128, NB, 4 * S], dt.bfloat16,
                         kind="ExternalInput")
    wpk_dr = nc.dram_tensor("wpack", [128, nw], dt.bfloat16,
                            kind="ExternalInput")
    fpk_dr = nc.dram_tensor("fpack", [128, nf], dt.float32,
                            kind="ExternalInput")
    ts_dr = nc.dram_tensor("tstore", [128, H, 1920], dt.bfloat16,
                           kind="ExternalInput")
    yout = nc.dram_tensor("y", [NB, E], dt.float32, kind="ExternalOutput")
    dbg = {}
    if KDBG:
        for nm, shp in [("xsT", [128, S]), ("xpT", [128, S]),
                        ("qT0", [128, S]), ("kT0", [128, S]),
                        ("vo", [128, JC * 8 * 32]), ("ut00", [128, 1024]),
                        ("osb00", [128, 512]),
                        ("oatt0", [128, 128]), ("att0", [128, 128]),
                        ("ffT", [128, S])]:
            dbg[nm] = nc.dram_tensor("dbg_" + nm, shp, dt.bfloat16,
                                     kind="ExternalOutput")

    with tile.TileContext(nc) as tc:
        import contextlib
        ctx = contextlib.ExitStack()
        with ctx:
            consts = ctx.enter_context(tc.tile_pool(name="consts", bufs=1))
            convp = ctx.enter_context(tc.tile_pool(name="convp", bufs=1))
            wpk = consts.tile([128, nw], dt.bfloat16, tag="wpack")
            nc.sync.dma_start(out=wpk[:, 0:128], in_=wpk_dr[:, 0:128])
            rhs8t = [convp.tile([128, 4 * S], dt.bfloat16, tag=f"rhs8b{b}",
                                name=f"rhs8b{b}")
                     for b in range(NB)]
            for b in range(NB):
                for hh in range(2):
                    nc.sync.dma_start(
                        out=rhs8t[b][:, hh * 2048:(hh + 1) * 2048],
                        in_=xin[:, b, hh * 2048:(hh + 1) * 2048])
            nc.sync.dma_start(out=wpk[:, 128:nw], in_=wpk_dr[:, 128:nw])
            fpk = consts.tile([128, nf], dt.float32, tag="fpack")
            nc.sync.dma_start(out=fpk, in_=fpk_dr[:])
            ts_sb = consts.tile([128, H, 1920], dt.bfloat16, tag="tstore")
            nc.sync.dma_start(out=ts_sb, in_=ts_dr[:])

            def W(name, rows=128):
                o, w = widx[name]
                return wpk[0:rows, o:o + w]

            def F(name, rows=128):
                o, w = fidx[name]
                return fpk[0:rows, o:o + w]

            w1c4_sb = W("w1c4")
            w2cT_sb = W("w2cT").rearrange("p (k e) -> p k e", k=16)
            scaleB_sb, biasB_sb = F("scaleB"), F("biasB")
            peT_sb = W("peT")
            wqT_sb = W("wqT").rearrange("p (g e) -> p g e", g=NG)
            wkT_sb = W("wkT").rearrange("p (g e) -> p g e", g=NG)
            wvT_sb = W("wvT")
            ffw1T_sb = W("ffw1T")
            ffb1_sb = F("ffb1")
            ffw2T_sb = W("ffw2T").rearrange("p (k e) -> p k e", k=4)
            ffb2_sb = F("ffb2")
            bcast4_sb = W("bcast4", rows=4)
            ident = W("ident")
            ones_sb = W("ones")
            eps_sb = F("eps")
            lnG_sb = F("lnG").rearrange("p (k e) -> p k e", k=3)
            lnB_sb = F("lnB").rearrange("p (k e) -> p k e", k=3)

            pers = ctx.enter_context(tc.tile_pool(name="pers", bufs=1))
            xpT = [pers.tile([128, S], dt.bfloat16, tag=f"xpT{b}",
                             name=f"xpT{b}") for b in range(NB)]
            xsrc = [pers.tile([128, SC, 128], dt.bfloat16, tag=f"xsrc{b}",
                              name=f"xsrc{b}") for b in range(NB)]
            oatt = [pers.tile([128, SC, 128], dt.bfloat16, tag=f"oatt{b}",
                              name=f"oatt{b}") for b in range(NB)]

            # =========== PHASE C: conv stem (Act table: Gelu) ===========
            xsTs = [None] * NB
            with tc.tile_pool(name="h1", bufs=1) as h1p, \
                 tc.tile_pool(name="cps", bufs=2, space="PSUM") as cps, \
                 tc.tile_pool(name="cp2", bufs=1, space="PSUM") as cp2, \
                 tc.tile_pool(name="cmisc", bufs=1) as cmisc:
                # conv2 accumulation interleaved into the conv1 loop: each
                # tap-slot's conv2 terms issue right after its gelu, so
                # conv2 (PE-only) hides under the Act gelu stream.
                for b in range(NB):
                    # h1all[:, n, cc, :] = gelu(conv1+bn) for tap-slot n,
                    # channel chunk cc; n = 2*r + sh
                    h1all = h1p.tile([128, 8, 4, 512], dt.bfloat16,
                                     tag="h1all", name="h1all")
                    ps2 = cp2.tile([128, 2, 512], dt.float32, tag="c2ps",
                                   name="c2ps")
                    for n in range(8):
                        r, sh = n // 2, n % 2
                        for half in range(2):
                            ps = cps.tile([128, 2, 512], dt.float32,
                                          tag="c1ps", name="c1ps")
                            for q in range(2):
                                cc = 2 * half + q
                                nc.tensor.matmul(
                                    ps[:, q, :],
                                    lhsT=w1c4_sb[32 * cc:32 * cc + 9, :],
                                    rhs=rhs8t[b][32 * cc:32 * cc + 9,
                                                 n * 512:(n + 1) * 512],
                                    start=True, stop=True,
                                    tile_position=(32 * cc, 0))
                            nc.scalar.activation(
                                h1all[:, n, 2 * half:2 * half + 2, :]
                                .rearrange("p a b -> p (a b)"),
                                ps.rearrange("p a b -> p (a b)"), AF.Gelu)
                        for cc in range(4):
                            nc.tensor.matmul(
                                ps2[:, sh, :],
                                lhsT=w2cT_sb[:, r * 4 + cc, :],
                                rhs=h1all[:, n, cc, :],
                                start=(r == 0 and cc == 0),
                                stop=(r == 3 and cc == 3),
                                skip_group_check=True)
                    xsT = cmisc.tile([128, S], dt.bfloat16, tag=f"xsT{b}",
                                     name=f"xsT{b}")
                    nc.scalar.activation(
                        xsT, ps2.rearrange("p a b -> p (a b)"),
                        AF.Gelu, bias=biasB_sb, scale=scaleB_sb)
                    xsTs[b] = xsT
                    if KDBG and b == 0:
                        nc.sync.dma_start(out=dbg["xsT"][:], in_=xsT)
                    nc.vector.tensor_tensor(xpT[b], xsT, peT_sb, op=ALU.add)
                    if KDBG and b == 0:
                        nc.sync.dma_start(out=dbg["xpT"][:], in_=xpT[b])

            # =========== PHASE Q: qkv projections (no Act) ===========
            qkv = ctx.enter_context(tc.tile_pool(name="qkv", bufs=1))
            qT = [[None] * NG for _ in range(NB)]
            kT = [[None] * NG for _ in range(NB)]
            VO = [None] * NB
            with tc.tile_pool(name="qps", bufs=4, space="PSUM") as qps, \
                 tc.tile_pool(name="vps", bufs=2, space="PSUM") as vps, \
                 tc.tile_pool(name="ctp", bufs=2, space="PSUM") as ctp:
                for b in range(NB):
                    for sc in range(SC):
                        ps = ctp.tile([128, 128], dt.bfloat16, tag="tps",
                                      name="tps")
                        nc.tensor.transpose(
                            ps, xsTs[b][:, sc * 128:(sc + 1) * 128], ident)
                        nc.vector.tensor_copy(out=xsrc[b][:, sc, :], in_=ps)
                    for g in range(NG):
                        qt = qkv.tile([128, S], dt.bfloat16, tag=f"qt{b}{g}",
                                      name=f"qt{b}{g}")
                        kt = qkv.tile([128, S], dt.bfloat16, tag=f"kt{b}{g}",
                                      name=f"kt{b}{g}")
                        for sh in range(2):
                            ps = qps.tile([128, 512], dt.float32, tag="qk",
                                          name="msq")
                            nc.tensor.matmul(
                                ps, lhsT=wqT_sb[:, g, :],
                                rhs=xpT[b][:, sh * 512:(sh + 1) * 512],
                                start=True, stop=True)
                            nc.scalar.activation(
                                qt[:, sh * 512:(sh + 1) * 512], ps, AF.Copy)
                            ps2 = qps.tile([128, 512], dt.float32, tag="qk",
                                           name="msk")
                            nc.tensor.matmul(
                                ps2, lhsT=wkT_sb[:, g, :],
                                rhs=xpT[b][:, sh * 512:(sh + 1) * 512],
                                start=True, stop=True)
                            nc.vector.tensor_copy(
                                out=kt[:, sh * 512:(sh + 1) * 512], in_=ps2)
                        if KDBG and b == 0 and g == KDBG_G:
                            nc.sync.dma_start(out=dbg["qT0"][:], in_=qt)
                            nc.sync.dma_start(out=dbg["kT0"][:], in_=kt)
                        qT[b][g] = qt
                        kT[b][g] = kt
                    # VO[:, jc, h, 0:16] = v head h; [..., 16] = 1.0
                    vo = qkv.tile([128, JC, 8, 32], dt.bfloat16,
                                  tag=f"vo{b}", name=f"vo{b}")
                    nc.vector.memset(vo[:, :, :, DH:DH + 1], 1.0)
                    for sc in range(SC):
                        ps = vps.tile([128, 128], dt.float32, tag="v",
                                      name="msv")
                        nc.tensor.matmul(
                            ps, lhsT=xpT[b][:, sc * 128:(sc + 1) * 128],
                            rhs=wvT_sb, start=True, stop=True)
                        nc.vector.tensor_copy(
                            out=vo[:, sc, :, 0:DH],
                            in_=ps.rearrange("p (h d) -> p h d", h=H))
                    VO[b] = vo
                    if KDBG and b == 0:
                        nc.sync.dma_start(
                            out=dbg["vo"][:],
                            in_=vo.rearrange("p a b c -> p (a b c)"))

            # =========== PHASE A: attention (Act table: Exp) ===========
            osbp = ctx.enter_context(tc.tile_pool(name="osbp", bufs=1))
            actx = contextlib.ExitStack()
            stp = actx.enter_context(
                tc.tile_pool(name="stp", bufs=2, space="PSUM"))
            pvp = actx.enter_context(
                tc.tile_pool(name="pvp", bufs=2, space="PSUM"))
            bvp = actx.enter_context(
                tc.tile_pool(name="bvp", bufs=2, space="PSUM"))
            utp = actx.enter_context(tc.tile_pool(name="utp", bufs=2))
            sm = actx.enter_context(tc.tile_pool(name="sm", bufs=2))
            osbs = [[None] * 4 for _ in range(NB)]

            for b in range(NB):
                for g in range(NG):
                    for ih in range(2):
                        i0 = ih * 512
                        pv = pvp.tile([128, 512], dt.float32, tag="pv",
                                      name="pv")
                        bv = bvp.tile([128, 512], dt.float32, tag="bv",
                                      name="bv")
                        uts = []
                        for jc in range(JC):
                            # scores for c-pairs (01), (23), each -> one exp
                            ut = utp.tile([128, 4, 512], dt.bfloat16,
                                          tag="ut", name="ut", bufs=4)
                            for half in range(2):
                                st = stp.tile([128, 2, 512], dt.float32,
                                              tag="st", name="st")
                                for q in range(2):
                                    c = 2 * half + q
                                    nc.tensor.matmul(
                                        st[:, q, :],
                                        lhsT=kT[b][g][32 * c:32 * c + DH,
                                                      jc * 128:(jc + 1) * 128],
                                        rhs=qT[b][g][32 * c:32 * c + DH,
                                                     i0:i0 + 512],
                                        start=True, stop=True,
                                        tile_position=(32 * c, 0))
                                nc.scalar.activation(
                                    ut.rearrange("p a b -> p (a b)")
                                    [:, half * 1024:(half + 1) * 1024],
                                    st.rearrange("p a b -> p (a b)"),
                                    AF.Exp, scale=SCALE)
                            if (KDBG and b == 0 and g == KDBG_G and ih == 0
                                    and jc == 0):
                                nc.sync.dma_start(
                                    out=dbg["ut00"][:],
                                    in_=ut.rearrange("p a b -> p (a b)")
                                    [:, 0:1024])
                            uts.append(ut)
                            # delay pv/bv by one jc so PE never waits on exp
                            if jc > 0:
                                _pv_bv(nc, pv, bv, VO[b], ts_sb, uts[jc - 1],
                                       g, jc - 1, i0, JC)
                        _pv_bv(nc, pv, bv, VO[b], ts_sb, uts[JC - 1],
                               g, JC - 1, i0, JC)

                        # drain PSUM to SBUF only; the whole softmax
                        # normalize/bias-add is deferred to phase L so the
                        # PE stream here is pure matmuls (no stalls between
                        # head groups).
                        o1 = osbp.tile([128, 512], dt.bfloat16,
                                       tag=f"o1_{b}_{g}_{ih}",
                                       name=f"o1_{b}_{g}_{ih}")
                        nc.vector.tensor_copy(out=o1, in_=pv)
                        bvc = osbp.tile([128, 512], dt.bfloat16,
                                        tag=f"bvc{b}_{g}_{ih}",
                                        name=f"bvc{b}_{g}_{ih}")
                        nc.vector.tensor_copy(out=bvc, in_=bv)
                        osbs[b][2 * g + ih] = (o1, bvc)

            actx.close()

            # =========== PHASE L: LN / FFN / GAP (Act: Sqrt) ===========
            lnp = ctx.enter_context(tc.tile_pool(name="lnp", bufs=2))
            msp = ctx.enter_context(
                tc.tile_pool(name="msp", bufs=2, space="PSUM"))
            gpp = ctx.enter_context(
                tc.tile_pool(name="gpp", bufs=1, space="PSUM"))
            att_p = ctx.enter_context(tc.tile_pool(name="attp", bufs=1))

            def layer_norm8(pairs, k):
                """Stage-major LayerNorm over the 8 [128,128] chunks of
                each (dst, src) [128,8,128] pair: one Act round trip for
                all chunks of all pairs."""
                nb = len(pairs)
                stats = lnp.tile([128, nb, 8, 6], dt.float32, tag="stats",
                                 name="stats")
                for i, (_, src) in enumerate(pairs):
                    for sc in range(SC):
                        nc.vector.bn_stats(out=stats[:, i, sc, :],
                                           in_=src[:, sc, :])
                mv = lnp.tile([128, nb, 16], dt.float32, tag="mv", name="mv")
                for i in range(nb):
                    for sc in range(SC):
                        nc.vector.bn_aggr(out=mv[:, i, 2 * sc:2 * sc + 2],
                                          in_=stats[:, i, sc, :])
                sd = lnp.tile([128, nb, 8], dt.float32, tag="sd",
                              name="sd")
                rstd = lnp.tile([128, nb, 8], dt.float32, tag="rstd",
                                name="rstd")
                for i in range(nb):
                    nc.scalar.activation(sd[:, i, :],
                                         mv[:, i, :].rearrange(
                                             "p (a b) -> p a b", b=2)[:, :, 1],
                                         AF.Sqrt, bias=eps_sb)
                    nc.vector.reciprocal(out=rstd[:, i, :], in_=sd[:, i, :])
                for i, (dst, src) in enumerate(pairs):
                    for sc in range(SC):
                        nc.vector.tensor_scalar(dst[:, sc, :], src[:, sc, :],
                                                mv[:, i, 2 * sc:2 * sc + 1],
                                                rstd[:, i, sc:sc + 1],
                                                ALU.subtract, ALU.mult)
                        if not ln_identity:
                            nc.vector.tensor_tensor(dst[:, sc, :],
                                                    dst[:, sc, :],
                                                    lnG_sb[:, k, :],
                                                    op=ALU.mult)
                            nc.vector.tensor_tensor(dst[:, sc, :],
                                                    dst[:, sc, :],
                                                    lnB_sb[:, k, :],
                                                    op=ALU.add)

            # deferred softmax finalize (stage-major over all 8 (b,g,ih)):
            # 1/Z broadcast via bcast4 matmul, normalize, add bias term
            rsbp = ctx.enter_context(
                tc.tile_pool(name="rsbp", bufs=2, space="PSUM"))
            osbf = [[None] * 4 for _ in range(NB)]
            for b in range(NB):
                for gih in range(4):
                    o1, bvc = osbs[b][gih]
                    rs = lnp.tile([4, 512], dt.bfloat16, tag="rs", name="rs")
                    nc.sync.dma_start(out=rs, in_=o1[DH::32, :])
                    rr = lnp.tile([4, 512], dt.bfloat16, tag="rr", name="rr")
                    with nc.allow_low_precision(reason="1/Z in bf16"):
                        nc.vector.reciprocal(out=rr, in_=rs)
                    rsb = rsbp.tile([128, 512], dt.float32, tag="rsb",
                                    name="rsb")
                    nc.tensor.matmul(rsb, lhsT=bcast4_sb, rhs=rr,
                                     start=True, stop=True)
                    o2 = lnp.tile([128, 512], dt.bfloat16, tag="o2",
                                  name="o2")
                    nc.vector.tensor_tensor(o2, o1, rsb, op=ALU.mult)
                    osb = lnp.tile([128, 512], dt.bfloat16, tag="osbf",
                                   name="osbf", bufs=8)
                    nc.vector.tensor_tensor(osb, o2, bvc, op=ALU.add)
                    osbf[b][gih] = osb
                    if KDBG and b == 0 and gih == 2 * KDBG_G:
                        nc.sync.dma_start(out=dbg["osb00"][:], in_=osb)
            # transpose sweep: osb [32c+d, i] -> oatt [i, h*16+d]
            for b in range(NB):
                for gih in range(4):
                    g, ih = gih // 2, gih % 2
                    for ic in range(4):
                        tp = msp.tile([128, 128], dt.bfloat16, tag="t1",
                                      name="tpo")
                        nc.tensor.transpose(
                            tp, osbf[b][gih][:, ic * 128:(ic + 1) * 128],
                            ident)
                        sc = ih * 4 + ic
                        tpr = tp.rearrange("p (c m) -> p c m", c=4)
                        nc.scalar.activation(
                            oatt[b].rearrange(
                                "p a (h d) -> p a h d",
                                h=H)[:, sc, 4 * g:4 * g + 4, :],
                            tpr[:, :, 0:DH], AF.Copy)
            if KDBG:
                nc.sync.dma_start(out=dbg["oatt0"][:], in_=oatt[0][:, 0, :])
            # stage-major across BOTH batches: each LN stage makes one Act
            # round trip for all 16 chunks; PE/DVE streams interleave.
            att = [att_p.tile([128, SC, 128], dt.bfloat16, tag=f"att{b}",
                              name=f"att{b}") for b in range(NB)]
            attT = [att_p.tile([128, S], dt.bfloat16, tag=f"attT{b}",
                               name=f"attT{b}") for b in range(NB)]
            o1 = [att_p.tile([128, SC, 128], dt.bfloat16, tag=f"o1{b}",
                             name=f"o1{b}") for b in range(NB)]
            layer_norm8(list(zip(o1, oatt)), 0)
            for b in range(NB):
                nc.vector.tensor_tensor(
                    o1[b].rearrange("p a b -> p (a b)"),
                    o1[b].rearrange("p a b -> p (a b)"),
                    xsrc[b].rearrange("p a b -> p (a b)"), op=ALU.add)
            layer_norm8(list(zip(att, o1)), 1)
            for b in range(NB):
                for sc in range(SC):
                    ps = msp.tile([128, 128], dt.bfloat16, tag="t1",
                                  name="msat")
                    nc.tensor.transpose(ps, att[b][:, sc, :], ident)
                    nc.scalar.activation(
                        attT[b][:, sc * 128:(sc + 1) * 128], ps, AF.Copy)
            if KDBG:
                nc.sync.dma_start(out=dbg["att0"][:], in_=att[0][:, 0, :])
            hrelu = [att_p.tile([128, 4, S], dt.bfloat16, tag=f"hr{b}",
                                name=f"hr{b}") for b in range(NB)]
            for b in range(NB):
                for fc in range(4):
                    for sh in range(2):
                        ps = msp.tile([128, 512], dt.float32, tag="f1",
                                      name="msf1")
                        nc.tensor.matmul(
                            ps, lhsT=ffw1T_sb[:, fc * 128:(fc + 1) * 128],
                            rhs=attT[b][:, sh * 512:(sh + 1) * 512],
                            start=True, stop=True)
                        nc.scalar.activation(
                            hrelu[b][:, fc, sh * 512:(sh + 1) * 512], ps,
                            AF.Relu, bias=ffb1_sb[:, fc:fc + 1])
            ffT = [att_p.tile([128, S], dt.bfloat16, tag=f"ffT{b}",
                              name=f"ffT{b}") for b in range(NB)]
            for b in range(NB):
                for sh in range(2):
                    ps = msp.tile([128, 512], dt.float32, tag="f1",
                                  name="msf2")
                    for fc in range(4):
                        nc.tensor.matmul(
                            ps, lhsT=ffw2T_sb[:, fc, :],
                            rhs=hrelu[b][:, fc, sh * 512:(sh + 1) * 512],
                            start=(fc == 0), stop=(fc == 3))
                    nc.scalar.activation(
                        ffT[b][:, sh * 512:(sh + 1) * 512], ps, AF.Identity,
                        bias=ffb2_sb)
            if KDBG:
                nc.sync.dma_start(out=dbg["ffT"][:], in_=ffT[0])
            l2in = [att_p.tile([128, SC, 128], dt.bfloat16, tag=f"l2in{b}",
                               name=f"l2in{b}") for b in range(NB)]
            for b in range(NB):
                for sc in range(SC):
                    ps = msp.tile([128, 128], dt.bfloat16, tag="t1",
                                  name="msft")
                    nc.tensor.transpose(ps, ffT[b][:, sc * 128:(sc + 1) * 128],
                                        ident)
                    nc.vector.tensor_tensor(l2in[b][:, sc, :],
                                            att[b][:, sc, :], ps, op=ALU.add)
            l2o = [att_p.tile([128, SC, 128], dt.bfloat16, tag=f"l2o{b}",
                              name=f"l2o{b}") for b in range(NB)]
            layer_norm8(list(zip(l2o, l2in)), 2)
            for b in range(NB):
                mps = gpp.tile([128, 1], dt.float32, tag=f"gap{b}",
                               name=f"gap{b}")
                for sc in range(SC):
                    nc.tensor.matmul(mps, lhsT=l2o[b][:, sc, :], rhs=ones_sb,
                                     start=(sc == 0), stop=(sc == SC - 1))
                ob = lnp.tile([128, 1], dt.float32, tag=f"ob{b}",
                              name=f"ob{b}")
                nc.scalar.activation(ob, mps, AF.Copy, scale=1.0 / S)
                nc.sync.dma_start(out=yout[b, :, None], in_=ob)

    nc.compile()
    return nc


def _pv_bv(nc, pv, bv, vo, ts_sb, ut, g, jc, i0, jcn):
    """Col-tiled pv (attn @ v with ones-column rowsum) and bv (rel-bias @ v)
    accumulation for one j-chunk."""
    for c in range(4):
        nc.tensor.matmul(
            pv[32 * c:32 * c + DH + 1, :],
            lhsT=vo[:, jc, 4 * g + c, 0:DH + 1],
            rhs=ut[:, c, :],
            start=(jc == 0), stop=(jc == jcn - 1),
            skip_group_check=True,
            tile_position=(0, 32 * c))
        nc.tensor.matmul(
            bv[32 * c:32 * c + DH, :],
            lhsT=vo[:, jc, 4 * g + c, 0:DH],
            rhs=ts_sb[:, 4 * g + c,
                      896 - 128 * jc + i0:896 - 128 * jc + i0 + 512],
            start=(jc == 0), stop=(jc == jcn - 1),
            skip_group_check=True,
            tile_position=(0, 32 * c))


_CACHE = {}


def _get_nc(host, widx, fidx, ln_identity):
    key = (ln_identity, host["wpack"].shape[1], host["fpack"].shape[1],
           KDBG, KDBG_G)
    if key not in _CACHE:
        _CACHE[key] = _build_bass(widx, fidx, host["wpack"].shape[1],
                                  host["fpack"].shape[1], ln_identity)
    return _CACHE[key]


def kernel(**inputs):
    inputs = {k: np.asarray(v) for k, v in inputs.items()}
    host, widx, fidx, ln_identity = _host_prep(inputs)
    nc = _get_nc(host, widx, fidx, ln_identity)

    from concourse.bass_utils import run_bass_kernel_spmd
    in_maps = _make_in_maps(inputs, host)
    try:
        res = run_bass_kernel_spmd(nc, in_maps, list(range(N_CORES)))
    except Exception:
        # transient NRT/device failures happen; one retry
        res = run_bass_kernel_spmd(nc, in_maps, list(range(N_CORES)))
    if KDBG:
        kernel.dbg = res.results[0]
    outs = [res.results[c]["y"] for c in range(N_CORES)]
    return np.concatenate(outs, axis=0).astype(F32)


def _make_in_maps(inputs, host):
    x = np.asarray(inputs["x"], dtype=F32)                 # [B, S, 4]
    xpad = np.zeros((B, S + 7, C_IN), F32)
    xpad[:, 3:S + 3, :] = x
    rhs8 = np.empty((B, 8, C_IN, S), F32)
    for t in range(8):
        rhs8[:, t] = xpad[:, t:t + S, :].transpose(0, 2, 1)
    rhs8 = rhs8.reshape(B, 8, C_IN * S)
    # replicate taps at partition offsets 0/32/64/96 + ones row at +8;
    # stored partition-major [128, NB, 4S] so the device DMA is contiguous
    rhs8r = np.zeros((B, 128, C_IN * S), BF16)
    for cc in range(4):
        rhs8r[:, 32 * cc:32 * cc + 8] = rhs8
        rhs8r[:, 32 * cc + 8] = 1.0
    in_maps = []
    for core in range(N_CORES):
        m = {"rhs8": np.ascontiguousarray(
            rhs8r[core * NB:(core + 1) * NB].transpose(1, 0, 2))}
        m.update(host)
        in_maps.append(m)
    return in_maps


def build(inputs):
    inputs = {k: np.asarray(v) for k, v in inputs.items()}
    host, widx, fidx, ln_identity = _host_prep(inputs)
    nc = _get_nc(host, widx, fidx, ln_identity)
    return nc, _make_in_maps(inputs, host)
